# revision 6
# baseline (speedup 1.0000x reference)
"""Self-contained Trainium2 kernel for nn_Linear_14293651161742.

Computes y = act_dequant(act_quant(x)) @ (weight * expand(w_scale))^T which is
mathematically y[m,o] = sum_k x[m,k] * weight[o,k] * w_scale[o//128, k//128]
(the act_quant divide/multiply round-trip is an exact no-op up to fp32
rounding, far below the bf16 matmul noise floor).

Strategy: shard M across the 8 cores (column of the hint is worse: it
replicates the 128 MiB x per core; M-sharding needs only 96 MiB/core of HBM
traffic, leaving the kernel compute-bound at the bf16 PE roofline).

Host does layout prep only (transposes / scale replication); all arithmetic
(dequant, casts, GEMM) runs on device.
"""

import sys

if "/opt/trn_rl_repo" not in sys.path:
    sys.path.insert(0, "/opt/trn_rl_repo")

import numpy as np

import concourse.bacc as bacc
import concourse.mybir as mybir
import concourse.tile as tile
from concourse import bass_utils

P = 128
N_CORES = 8

F32 = mybir.dt.float32
BF16 = mybir.dt.bfloat16


def build_gemm_nc(M_loc: int, K: int, O: int):
    """Per-core program: yt[O, M_loc] = (wt * scale)^T-contracted with xt.

    Inputs (per core):
      xt  [K, M_loc] f32 : x slice, K-major (pre-transposed on host)
      wt  [K, O]     f32 : full weight, K-major (pre-transposed on host)
      ws  [P, K//P, O//P] f32 : w_scale[ob, kb] replicated across partitions,
                                indexed [p, kb, ob]
    Output:
      yt  [O, M_loc] f32 : y^T slice (host transposes back)
    """
    KT = K // P            # k tiles
    OB = O // P            # 128-wide o tiles
    OCW = 256              # o-chunk width (psum partition groups per chunk: OCW/P)
    OC = O // OCW          # o chunks
    JT = OCW // P          # o tiles per chunk
    MCW = min(512, M_loc)  # matmul moving free dim
    MC = M_loc // MCW      # m chunks

    nc = bacc.Bacc("TRN2", target_bir_lowering=False, debug=False)
    xt = nc.dram_tensor("xt", [K, M_loc], F32, kind="ExternalInput")
    wt = nc.dram_tensor("wt", [K, O], F32, kind="ExternalInput")
    ws = nc.dram_tensor("ws", [P, KT, OB], F32, kind="ExternalInput")
    yt = nc.dram_tensor("yt", [O, M_loc], F32, kind="ExternalOutput")

    xt_r = xt.ap().rearrange("(kt p) m -> p kt m", p=P)    # [P, KT, M_loc]
    wt_r = wt.ap().rearrange("(kt p) o -> p kt o", p=P)    # [P, KT, O]
    yt_r = yt.ap().rearrange("(ot p) m -> p ot m", p=P)    # [P, OB, M_loc]

    WB = 2 if KT % 2 == 0 else 1     # k-tiles per w staging DMA

    # Round schedule: round 0 covers two chunks (PE gets 2x work per arriving
    # k-tile while x streams in); later rounds one chunk each, with PSUM
    # double-buffered (psum tags have bufs=2) so chunk transitions never wait
    # on evictions.
    rounds = [[0, 1]] + [[oc] for oc in range(2, OC)] if OC >= 2 else [[0]]

    with tile.TileContext(nc) as tc:
        with (
            tc.tile_pool(name="const", bufs=1) as const_pool,
            tc.tile_pool(name="xbf", bufs=1) as xbf_pool,
            tc.tile_pool(name="wstage", bufs=8) as wstage_pool,
            tc.tile_pool(name="wbf", bufs=2) as wbf_pool,
            tc.tile_pool(name="yout", bufs=2) as y_pool,
            tc.tile_pool(name="psum", bufs=2, space="PSUM") as psum_pool,
        ):
            ws_sb = const_pool.tile([P, KT, OB], F32)
            nc.sync.dma_start(ws_sb[:], ws.ap())

            x_bf = [None] * KT
            w_chunks = {}  # oc -> list of KT bf16 [P, OCW] tiles

            def emit_x_load(kt):
                # SWDGE dma casts f32->bf16 inline; runs on a separate queue
                # concurrent with the HWDGE w loads.
                xb = xbf_pool.tile([P, M_loc], BF16, tag=f"xb{kt}",
                                   name=f"xb{kt}")
                nc.gpsimd.dma_start(xb[:], xt_r[:, kt, :])
                x_bf[kt] = xb

            def emit_w_load(oc, g):
                wst = wstage_pool.tile([P, WB, OCW], F32, tag="wst", name="wst")
                nc.sync.dma_start(
                    wst[:], wt_r[:, g * WB:(g + 1) * WB, oc * OCW:(oc + 1) * OCW]
                )
                for i in range(WB):
                    kt = g * WB + i
                    wb = wbf_pool.tile([P, OCW], BF16, tag=f"wb{kt}",
                                       name=f"wb{kt}")
                    nc.vector.tensor_tensor(
                        wb.rearrange("p (g j) -> p g j", j=P),
                        wst[:, i].rearrange("p (g j) -> p g j", j=P),
                        ws_sb[:, kt, oc * JT:(oc + 1) * JT, None].to_broadcast(
                            [P, JT, P]
                        ),
                        mybir.AluOpType.mult,
                    )
                    w_chunks[oc][kt] = wb

            # Prologue: round-0 w chunks on HWDGE, x on SWDGE, interleaved so
            # low k-tiles of everything arrive first.
            for oc in rounds[0]:
                w_chunks[oc] = [None] * KT
            for g in range(KT // WB):
                for oc in rounds[0]:
                    emit_w_load(oc, g)
                for i in range(WB):
                    emit_x_load(g * WB + i)

            next_chunk = rounds[0][-1] + 1
            for rnd in rounds:
                # prefetch upcoming chunks ahead of this round's matmuls in
                # program order (SP queue: never behind compute-gated work)
                n_pre = len(rnd)
                for _ in range(n_pre):
                    if next_chunk < OC:
                        w_chunks[next_chunk] = [None] * KT
                        for g in range(KT // WB):
                            emit_w_load(next_chunk, g)
                        next_chunk += 1
                psums = {}
                for oc in rnd:
                    for j in range(JT):
                        for mc in range(MC):
                            psums[(oc, j, mc)] = psum_pool.tile(
                                [P, MCW], F32, tag=f"ps{j}_{mc}",
                                name=f"ps{j}_{mc}"
                            )
                for kt in range(KT):
                    for oc in rnd:
                        for j in range(JT):
                            lhsT = w_chunks[oc][kt][:, j * P:(j + 1) * P]
                            for mc in range(MC):
                                nc.tensor.matmul(
                                    psums[(oc, j, mc)][:],
                                    lhsT,
                                    x_bf[kt][:, mc * MCW:(mc + 1) * MCW],
                                    start=(kt == 0),
                                    stop=(kt == KT - 1),
                                )
                # evict on DVE (fast); gather per (oc, mc), store on ACT ring
                for oc in rnd:
                    for mc in range(MC):
                        ysb = y_pool.tile([P, JT, MCW], F32, tag=f"ysb{mc}",
                                          name=f"ysb{mc}")
                        for j in range(JT):
                            nc.vector.tensor_copy(ysb[:, j],
                                                  psums[(oc, j, mc)][:])
                        nc.scalar.dma_start(
                            yt_r[:, oc * JT:(oc + 1) * JT,
                                 mc * MCW:(mc + 1) * MCW],
                            ysb[:],
                        )
                    del w_chunks[oc]
    nc.compile()
    return nc


_CACHED = {}


def _get_nc(M_loc, K, O):
    key = (M_loc, K, O)
    if key not in _CACHED:
        _CACHED[key] = build_gemm_nc(M_loc, K, O)
    return _CACHED[key]


def kernel(x: np.ndarray, weight: np.ndarray, w_scale: np.ndarray) -> np.ndarray:
    M, K = x.shape
    O = weight.shape[0]
    assert M % N_CORES == 0
    M_loc = M // N_CORES
    KT, OB = K // P, O // P

    nc = _get_nc(M_loc, K, O)

    wt = np.ascontiguousarray(weight.T)                       # [K, O]
    ws_rep = np.ascontiguousarray(
        np.broadcast_to(w_scale.T[None], (P, KT, OB))
    ).astype(np.float32)

    in_maps = []
    for c in range(N_CORES):
        xt_c = np.ascontiguousarray(x[c * M_loc:(c + 1) * M_loc, :].T)  # [K, M_loc]
        in_maps.append({"xt": xt_c, "wt": wt, "ws": ws_rep})

    res = bass_utils.run_bass_kernel_spmd(
        nc, in_maps, core_ids=list(range(N_CORES))
    )
    return np.concatenate(
        [np.ascontiguousarray(res.results[c]["yt"].T) for c in range(N_CORES)],
        axis=0,
    )
